# revision 1
# baseline (speedup 1.0000x reference)
"""LeNet-style ClientNet (dense_cnn) on 8 Trainium2 NeuronCores.

Strategy (data-parallel, batch sharded 8x1024):
  host: ps-weighted average of the 16 client stacks (tiny einsum), weights
        pre-shaped into banded lhsT layouts for the PE.
  core: conv1 as one K=51 matmul per N-block (50 = 10 relrow x 5 dx banded
        rows + ones row carrying the bias), relu+maxpool fused on DVE/GPSIMD,
        conv2 as 5 dx-accumulated K=120 matmuls (20 cin x 6 relrow), fc1 as
        16 accumulated K=50 matmuls (one per spatial tap), fc2 K=125 x4.
        All matmuls float32r (full PE rate, ~tf32 accuracy), psum fp32.
"""

import contextlib
import sys

import numpy as np

sys.path.insert(0, "/opt/trn_rl_repo")

import concourse.bass as bass  # noqa: E402
import concourse.bacc as bacc  # noqa: E402
import concourse.mybir as mybir  # noqa: E402
from concourse.tile import TileContext  # noqa: E402

F32R = mybir.dt.float32r
F32 = mybir.dt.float32
MAX = mybir.AluOpType.max
ADD = mybir.AluOpType.add

NCORES = 8
BC = 1024            # samples per core
CH = 32              # samples per chunk
NCH = BC // CH       # 32 chunks
QC = 8               # chunks per fc group (256 samples)
NQ = NCH // QC       # 4 fc groups


def _ap(t, off, dims):
    return bass.AP(tensor=t.tensor, offset=t.offset + off, ap=[list(d) for d in dims])


def _pitch(t):
    return t.ap[0][0]


def build_host_weights(ps, conv1_w, conv1_b, conv2_w, conv2_b,
                       fc1_w, fc1_b, fc2_w, fc2_b):
    ps = np.asarray(ps, np.float64)
    W1 = np.einsum("n,noihw->oihw", ps, np.asarray(conv1_w, np.float64))[:, 0]  # [20,5,5]
    b1 = ps @ np.asarray(conv1_b, np.float64)                                   # [20]
    W2 = np.einsum("n,noihw->oihw", ps, np.asarray(conv2_w, np.float64))        # [50,20,5,5]
    b2 = ps @ np.asarray(conv2_b, np.float64)                                   # [50]
    Wf1 = np.einsum("n,nof->of", ps, np.asarray(fc1_w, np.float64))             # [500,800]
    bf1 = ps @ np.asarray(fc1_b, np.float64)                                    # [500]
    Wf2 = np.einsum("n,nof->of", ps, np.asarray(fc2_w, np.float64))             # [10,500]
    bf2 = ps @ np.asarray(fc2_b, np.float64)                                    # [10]

    # conv1 lhsT [51, 120]: k = dx*10 + relrow (rows 0..49), row 50 = bias.
    # m = e*60 + o*3 + t ; output row y = 6g + 2t + e ; input row 6g + relrow,
    # dy = relrow - (2t + e) in 0..4.
    # conv1 lhsT [41, 104]: k = dx*8 + rr (rows 0..39), row 40 = bias ones-row.
    # m = e*64 + u*20 + o ; out row y = 4G + 2u + e ; input row 4G + rr,
    # dy = rr - (2u + e) in 0..4.
    L1 = np.zeros((41, 104), np.float32)
    for dx in range(5):
        for rr in range(8):
            for e in range(2):
                for u in range(2):
                    for o in range(20):
                        dy = rr - (2 * u + e)
                        if 0 <= dy <= 4:
                            L1[dx * 8 + rr, e * 64 + u * 20 + o] = W1[o, dy, dx]
    for e in range(2):
        for u in range(2):
            for o in range(20):
                L1[40, e * 64 + u * 20 + o] = b1[o]

    # conv2 lhsT [120, 5*100]: k = c*6 + relrow, m(dx) = dx*100 + e*50 + o.
    # out row y' = 2gg + e ; pooled input row 2gg + relrow ; dy = relrow - e.
    L2 = np.zeros((121, 570), np.float32)
    for dx in range(5):
        for c in range(20):
            for rr in range(6):
                for e in range(2):
                    dy = rr - e
                    if 0 <= dy <= 4:
                        L2[rr * 20 + c, dx * 114 + e * 64:dx * 114 + e * 64 + 50] = \
                            W2[:, c, dy, dx]
    for e in range(2):
        L2[120, e * 64:e * 64 + 50] = b2

    # fc1 lhsT [50, 16*500]: tap f = gg*4 + xp; torch feature id = o*16 + f.
    LF1 = np.zeros((51, 16 * 500), np.float32)
    for gg in range(4):
        for xp in range(4):
            f = gg * 4 + xp
            for o in range(50):
                LF1[o, f * 500:(f + 1) * 500] = Wf1[:, o * 16 + f]
    LF1[50, 0:500] = bf1

    # fc2 lhsT [125, 4*10]
    LF2 = np.zeros((126, 40), np.float32)
    for c in range(4):
        LF2[0:125, c * 10:(c + 1) * 10] = Wf2[:, c * 125:(c + 1) * 125].T
    LF2[125, 0:10] = bf2

    return dict(
        lhsT1=L1,
        lhsT2=L2.astype(np.float32),
        lf1=LF1.astype(np.float32),
        lf2=LF2.astype(np.float32),
        onesv=np.ones((4096,), np.float32),
    )


def stage_x(xc):
    """Host-side im2col-lite: [BC,784] -> [NCH, 41, CH*144] staged conv1 rhs."""
    x3 = np.asarray(xc, np.float32).reshape(NCH, CH, 28, 28)
    st = np.empty((NCH, 41, CH, 144), np.float32)
    st[:, 40] = 1.0
    rows_base = 4 * np.arange(6)
    for dx in range(5):
        for rr in range(8):
            k = dx * 8 + rr
            rows = rows_base + rr
            st[:, k] = x3[:, :, rows, :][:, :, :, dx:dx + 24].reshape(NCH, CH, 144)
    return st.reshape(NCH, 41, CH * 144)


def build_nc():
    nc = bacc.Bacc()
    x_d = nc.dram_tensor("x", [NCH, 41, CH * 144], F32R, kind="ExternalInput")
    L1_d = nc.dram_tensor("lhsT1", [41, 104], F32R, kind="ExternalInput")
    L2_d = nc.dram_tensor("lhsT2", [121, 570], F32R, kind="ExternalInput")
    LF1_d = nc.dram_tensor("lf1", [51, 8000], F32R, kind="ExternalInput")
    LF2_d = nc.dram_tensor("lf2", [126, 40], F32R, kind="ExternalInput")
    ON_d = nc.dram_tensor("onesv", [4096], F32R, kind="ExternalInput")
    out_d = nc.dram_tensor("out", [BC, 10], F32, kind="ExternalOutput")

    ctx = contextlib.ExitStack()
    with ctx:
        with TileContext(nc) as tc:
            with contextlib.ExitStack() as pctx:
                cpool = pctx.enter_context(tc.tile_pool(name="const", bufs=1))
                r1p = pctx.enter_context(tc.tile_pool(name="r1", bufs=2))
                p1p = pctx.enter_context(tc.tile_pool(name="p1", bufs=2))
                y1p = pctx.enter_context(tc.tile_pool(name="y1", bufs=2))
                c2rp = pctx.enter_context(tc.tile_pool(name="c2r", bufs=2))
                p2p = pctx.enter_context(tc.tile_pool(name="p2", bufs=2))
                t2p = pctx.enter_context(tc.tile_pool(name="t2", bufs=2))
                y2p = pctx.enter_context(tc.tile_pool(name="y2", bufs=2))
                y3p = pctx.enter_context(tc.tile_pool(name="y3", bufs=2))
                osbp = pctx.enter_context(tc.tile_pool(name="osb", bufs=2))
                e1p = pctx.enter_context(tc.tile_pool(name="e1", bufs=2))
                p1bp = pctx.enter_context(tc.tile_pool(name="p1b", bufs=2))
                p2bp = pctx.enter_context(tc.tile_pool(name="p2b", bufs=2))
                e2p = pctx.enter_context(tc.tile_pool(name="e2", bufs=2))
                ps1p = pctx.enter_context(tc.tile_pool(name="ps1", bufs=2, space="PSUM"))
                ps2p = pctx.enter_context(tc.tile_pool(name="ps2", bufs=2, space="PSUM"))
                ps3p = pctx.enter_context(tc.tile_pool(name="ps3", bufs=2, space="PSUM"))
                ps4p = pctx.enter_context(tc.tile_pool(name="ps4", bufs=2, space="PSUM"))
                # --- constants ---
                L1 = cpool.tile([41, 104], F32R)
                nc.sync.dma_start(out=L1[:, :], in_=L1_d[:, :])
                L2 = cpool.tile([121, 570], F32R)
                nc.sync.dma_start(out=L2[:, :], in_=L2_d[:, :])
                LF1 = cpool.tile([51, 8000], F32R)
                nc.sync.dma_start(out=LF1[:, :], in_=LF1_d[:, :])
                LF2 = cpool.tile([126, 40], F32R)
                nc.sync.dma_start(out=LF2[:, :], in_=LF2_d[:, :])

                y2_cur = None
                c2r_tiles = []
                for j in range(2):
                    t_ = c2rp.tile([121, CH * 48], F32R)
                    nc.sync.dma_start(
                        out=_ap(t_[:, :], 120 * _pitch(t_[:, :]),
                                [[_pitch(t_[:, :]), 1], [1, CH * 48]]),
                        in_=_ap(ON_d[:], 0, [[0, 1], [1, CH * 48]]),
                    )
                    c2r_tiles.append(t_)
                for i in range(NCH):
                    q = i // QC
                    # ---- conv1 rhs: host-staged, one DMA ----
                    R1 = r1p.tile([41, CH * 144], F32R)
                    pr = _pitch(R1[:, :])
                    nc.sync.dma_start(out=R1[:, :], in_=x_d[i, :, :])
                    # ---- conv1 matmuls + evict + pool-x ----
                    P1 = p1p.tile([104, CH * 72], F32R)
                    pp1 = _pitch(P1[:, :])
                    for bs in range(CH // 2):
                        ps1 = ps1p.tile([104, 288], F32)
                        nc.tensor.matmul(
                            ps1[:, :], L1[:, :],
                            _ap(R1[:, :], bs * 288, [[pr, 41], [1, 288]]),
                            start=True, stop=True,
                        )
                        E1 = e1p.tile([104, 288], F32)
                        pe1 = _pitch(E1[:, :])
                        nc.scalar.copy(out=E1[:, :], in_=ps1[:, :])
                        nc.vector.tensor_tensor(
                            out=_ap(P1[:, :], bs * 144,
                                    [[pp1, 104], [72, 2], [12, 6], [1, 12]]),
                            in0=_ap(E1[:, :], 0,
                                    [[pe1, 104], [144, 2], [24, 6], [2, 12]]),
                            in1=_ap(E1[:, :], 1,
                                    [[pe1, 104], [144, 2], [24, 6], [2, 12]]),
                            op=MAX,
                        )
                    # ---- conv1 pool-y + relu ----
                    P1B = p1bp.tile([40, CH * 72], F32R)
                    nc.sync.dma_start(out=P1B[:, :], in_=P1[64:104, :])
                    Y1 = y1p.tile([40, CH * 72], F32R)
                    nc.vector.tensor_tensor(
                        out=Y1[:, :], in0=P1[0:40, :], in1=P1B[:, :], op=MAX)
                    nc.vector.tensor_scalar_max(out=Y1[:, :], in0=Y1[:, :],
                                                scalar1=0.0)
                    # ---- shuffle Y1 -> C2R (6 DMAs) ----
                    C2R = c2r_tiles[i % 2]
                    pc = _pitch(C2R[:, :])
                    py1 = _pitch(Y1[:, :])
                    for u in range(2):
                        for v in range(3):
                            nc.sync.dma_start(
                                out=_ap(C2R[:, :], (2 * v + u) * 20 * pc,
                                        [[pc, 20], [48, CH], [1, 48]]),
                                in_=_ap(Y1[:, :], u * 20 * py1 + v * 12,
                                        [[py1, 20], [72, CH], [1, 48]]),
                            )
                    # ---- conv2: groups of 16 samples ----
                    P2 = p2p.tile([114, CH * 16], F32R)
                    pp2 = _pitch(P2[:, :])
                    for bg in range(CH // 16):
                        ps2 = ps2p.tile([114, 512], F32)
                        pq = _pitch(ps2[:, :])
                        for dx in range(5):
                            nc.tensor.matmul(
                                ps2[:, :],
                                _ap(L2[:, :], dx * 114,
                                    [[_pitch(L2[:, :]), 121], [1, 114]]),
                                _ap(C2R[:, :], bg * 16 * 48 + dx,
                                    [[pc, 121], [48, 16], [12, 4], [1, 8]]),
                                start=(dx == 0), stop=(dx == 4),
                            )
                        E2 = e2p.tile([114, 512], F32)
                        pe2 = _pitch(E2[:, :])
                        nc.scalar.copy(out=E2[:, :], in_=ps2[:, :])
                        nc.vector.tensor_tensor(
                            out=_ap(P2[:, :], bg * 256,
                                    [[pp2, 114], [16, 16], [4, 4], [1, 4]]),
                            in0=_ap(E2[:, :], 0,
                                    [[pe2, 114], [32, 16], [8, 4], [2, 4]]),
                            in1=_ap(E2[:, :], 1,
                                    [[pe2, 114], [32, 16], [8, 4], [2, 4]]),
                            op=MAX,
                        )
                    # ---- conv2 pool-y (gpsimd) + bias/relu into Y2 ----
                    P2B = p2bp.tile([50, CH * 16], F32R)
                    nc.sync.dma_start(out=P2B[:, :], in_=P2[64:114, :])
                    T2 = t2p.tile([50, CH * 16], F32R)
                    nc.vector.tensor_tensor(
                        out=T2[:, :], in0=P2[0:50, :], in1=P2B[:, :], op=MAX)
                    if i % QC == 0:
                        y2_cur = y2p.tile([51, QC * CH * 16], F32R)
                        nc.sync.dma_start(
                            out=_ap(y2_cur[:, :], 50 * _pitch(y2_cur[:, :]),
                                    [[_pitch(y2_cur[:, :]), 1], [1, QC * CH * 16]]),
                            in_=_ap(ON_d[:], 0, [[0, 1], [1, QC * CH * 16]]),
                        )
                    Y2 = y2_cur
                    nc.vector.tensor_scalar_max(
                        out=Y2[0:50, (i % QC) * CH * 16:(i % QC + 1) * CH * 16],
                        in0=T2[:, :], scalar1=0.0,
                    )
                    # ---- fc1 + fc2 per completed 256-sample group ----
                    if i % QC == QC - 1:
                        NB = QC * CH  # 256
                        py2 = _pitch(Y2[:, :])
                        Y3 = y3p.tile([126, 4 * NB], F32R)
                        nc.sync.dma_start(
                            out=_ap(Y3[:, :], 125 * _pitch(Y3[:, :]),
                                    [[_pitch(Y3[:, :]), 1], [1, 4 * NB]]),
                            in_=_ap(ON_d[:], 0, [[0, 1], [1, 4 * NB]]),
                        )
                        for c in range(4):
                            ps3 = ps3p.tile([125, NB], F32)
                            for f in range(16):
                                nc.tensor.matmul(
                                    ps3[:, :],
                                    _ap(LF1[:, :], f * 500 + c * 125,
                                        [[_pitch(LF1[:, :]), 51], [1, 125]]),
                                    _ap(Y2[:, :], f, [[py2, 51], [16, NB]]),
                                    start=(f == 0), stop=(f == 15),
                                )
                            nc.vector.tensor_scalar_max(
                                out=Y3[0:125, c * NB:(c + 1) * NB],
                                in0=ps3[:, :], scalar1=0.0,
                            )
                        ps4 = ps4p.tile([10, NB], F32)
                        for c in range(4):
                            nc.tensor.matmul(
                                ps4[:, :],
                                _ap(LF2[:, :], c * 10,
                                    [[_pitch(LF2[:, :]), 126], [1, 10]]),
                                _ap(Y3[:, :], c * NB,
                                    [[_pitch(Y3[:, :]), 126], [1, NB]]),
                                start=(c == 0), stop=(c == 3),
                            )
                        OUT = osbp.tile([10, NB], F32)
                        nc.vector.tensor_copy(out=OUT[:, :], in_=ps4[:, :])
                        nc.sync.dma_start(
                            out=_ap(out_d[:], q * NB * 10, [[1, 10], [10, NB]]),
                            in_=_ap(OUT[:, :], 0, [[_pitch(OUT[:, :]), 10], [1, NB]]),
                        )
    return nc


_NC_CACHE = None


def kernel(x, ps, conv1_w, conv1_b, conv2_w, conv2_b, fc1_w, fc1_b, fc2_w, fc2_b):
    global _NC_CACHE
    from concourse import bass_utils

    w = build_host_weights(ps, conv1_w, conv1_b, conv2_w, conv2_b,
                           fc1_w, fc1_b, fc2_w, fc2_b)
    if _NC_CACHE is None:
        _NC_CACHE = build_nc()
        _NC_CACHE.finalize()
    nc = _NC_CACHE

    x = np.asarray(x, np.float32).reshape(8192, 784)
    in_maps = []
    for c in range(NCORES):
        m = dict(w)
        m["x"] = stage_x(x[c * BC:(c + 1) * BC])
        in_maps.append(m)
    res = bass_utils.run_bass_kernel_spmd(nc, in_maps, core_ids=list(range(NCORES)))
    out = np.concatenate([r["out"] for r in res.results], axis=0)
    return out.astype(np.float32)



# revision 2
# speedup vs baseline: 6.8524x; 6.8524x over previous
"""LeNet-style ClientNet (dense_cnn) on 8 Trainium2 NeuronCores.

Strategy (data-parallel, batch sharded 8x1024):
  host: ps-weighted average of the 16 client stacks (tiny einsum), weights
        pre-shaped into banded lhsT layouts for the PE. x shipped raw as
        fp16 [1024,784] per core (no host im2col -> 15x less axon traffic).
  core: on-device im2col-lite: per 32-sample chunk, 6 strided DMAs stage
        x rows into XS[9, CH*168] fp16 (8 rr-bands + ones row). conv1 is
        5 dx-accumulated K=9 fp16 matmuls per 2-sample group, relu+maxpool
        fused on DVE, conv2 as 5 dx-accumulated K=121 matmuls, fc1 as 16
        accumulated K=51 matmuls (one per spatial tap), fc2 K=126 x4.
        conv2/fc weights ship fp16 and are cast once on-device to f32r.
"""

import contextlib
import sys

import numpy as np

sys.path.insert(0, "/opt/trn_rl_repo")

import concourse.bass as bass  # noqa: E402
import concourse.bacc as bacc  # noqa: E402
import concourse.mybir as mybir  # noqa: E402
from concourse.tile import TileContext  # noqa: E402

F32R = mybir.dt.float32r
F32 = mybir.dt.float32
F16 = mybir.dt.float16
MAX = mybir.AluOpType.max
ADD = mybir.AluOpType.add

NCORES = 8
BC = 1024            # samples per core
CH = 32              # samples per chunk
NCH = BC // CH       # 32 chunks
QC = 8               # chunks per fc group (256 samples)
NQ = NCH // QC       # 4 fc groups


def _ap(t, off, dims):
    return bass.AP(tensor=t.tensor, offset=t.offset + off, ap=[list(d) for d in dims])


def _pitch(t):
    return t.ap[0][0]


def build_host_weights(ps, conv1_w, conv1_b, conv2_w, conv2_b,
                       fc1_w, fc1_b, fc2_w, fc2_b):
    ps = np.asarray(ps, np.float64)
    W1 = np.einsum("n,noihw->oihw", ps, np.asarray(conv1_w, np.float64))[:, 0]  # [20,5,5]
    b1 = ps @ np.asarray(conv1_b, np.float64)                                   # [20]
    W2 = np.einsum("n,noihw->oihw", ps, np.asarray(conv2_w, np.float64))        # [50,20,5,5]
    b2 = ps @ np.asarray(conv2_b, np.float64)                                   # [50]
    Wf1 = np.einsum("n,nof->of", ps, np.asarray(fc1_w, np.float64))             # [500,800]
    bf1 = ps @ np.asarray(fc1_b, np.float64)                                    # [500]
    Wf2 = np.einsum("n,nof->of", ps, np.asarray(fc2_w, np.float64))             # [10,500]
    bf2 = ps @ np.asarray(fc2_b, np.float64)                                    # [10]

    # conv1 lhsT [9, 5*104]: per dx a [9, 104] block; k rows 0..7 = rr bands,
    # row 8 = bias ones-row (dx=0 block only). m = e*64 + u*20 + o ;
    # out row y = 4G + 2u + e ; input row 4G + rr ; dy = rr - (2u + e) in 0..4.
    # The dx column shift lives in the rhs AP offset, not the weights.
    L1 = np.zeros((9, 520), np.float32)
    for dx in range(5):
        for rr in range(8):
            for e in range(2):
                for u in range(2):
                    for o in range(20):
                        dy = rr - (2 * u + e)
                        if 0 <= dy <= 4:
                            L1[rr, dx * 104 + e * 64 + u * 20 + o] = W1[o, dy, dx]
    for e in range(2):
        for u in range(2):
            for o in range(20):
                L1[8, e * 64 + u * 20 + o] = b1[o]

    # conv2 lhsT [121, 5*114]: k = rr*20 + c, m(dx) = dx*114 + e*64 + o.
    # out row y' = 2gg + e ; pooled input row 2gg + rr ; dy = rr - e.
    L2 = np.zeros((121, 570), np.float32)
    for dx in range(5):
        for c in range(20):
            for rr in range(6):
                for e in range(2):
                    dy = rr - e
                    if 0 <= dy <= 4:
                        L2[rr * 20 + c, dx * 114 + e * 64:dx * 114 + e * 64 + 50] = \
                            W2[:, c, dy, dx]
    for e in range(2):
        L2[120, e * 64:e * 64 + 50] = b2

    # fc1 lhsT [51, 16*500]: tap f = gg*4 + xp; torch feature id = o*16 + f.
    LF1 = np.zeros((51, 16 * 500), np.float32)
    for gg in range(4):
        for xp in range(4):
            f = gg * 4 + xp
            for o in range(50):
                LF1[o, f * 500:(f + 1) * 500] = Wf1[:, o * 16 + f]
    LF1[50, 0:500] = bf1

    # fc2 lhsT [125, 4*10]
    LF2 = np.zeros((126, 40), np.float32)
    for c in range(4):
        LF2[0:125, c * 10:(c + 1) * 10] = Wf2[:, c * 125:(c + 1) * 125].T
    LF2[125, 0:10] = bf2

    return dict(
        l1=L1.astype(np.float16),
        l2h=L2.astype(np.float16),
        lf1h=LF1.astype(np.float16),
        lf2=LF2.astype(np.float32),
        onesv=np.ones((4096,), np.float32),
        onesh=np.ones((CH * 168,), np.float16),
    )


def build_in_maps(x, ps, conv1_w, conv1_b, conv2_w, conv2_b,
                  fc1_w, fc1_b, fc2_w, fc2_b):
    w = build_host_weights(ps, conv1_w, conv1_b, conv2_w, conv2_b,
                           fc1_w, fc1_b, fc2_w, fc2_b)
    xh = np.ascontiguousarray(
        np.asarray(x, np.float32).reshape(NCORES, BC, 784).astype(np.float16))
    in_maps = []
    for c in range(NCORES):
        m = dict(w)
        m["x"] = xh[c]
        in_maps.append(m)
    return in_maps


def build_nc():
    nc = bacc.Bacc()
    x_d = nc.dram_tensor("x", [BC, 784], F16, kind="ExternalInput")
    L1_d = nc.dram_tensor("l1", [9, 520], F16, kind="ExternalInput")
    L2_d = nc.dram_tensor("l2h", [121, 570], F16, kind="ExternalInput")
    LF1_d = nc.dram_tensor("lf1h", [51, 8000], F16, kind="ExternalInput")
    LF2_d = nc.dram_tensor("lf2", [126, 40], F32R, kind="ExternalInput")
    ON_d = nc.dram_tensor("onesv", [4096], F32R, kind="ExternalInput")
    ONH_d = nc.dram_tensor("onesh", [CH * 168], F16, kind="ExternalInput")
    out_d = nc.dram_tensor("out", [BC, 10], F32, kind="ExternalOutput")

    ctx = contextlib.ExitStack()
    with ctx:
        with TileContext(nc) as tc:
            with contextlib.ExitStack() as pctx:
                cpool = pctx.enter_context(tc.tile_pool(name="const", bufs=1))
                xsp = pctx.enter_context(tc.tile_pool(name="xs", bufs=2))
                p1p = pctx.enter_context(tc.tile_pool(name="p1", bufs=2))
                y1p = pctx.enter_context(tc.tile_pool(name="y1", bufs=2))
                c2rp = pctx.enter_context(tc.tile_pool(name="c2r", bufs=2))
                p2p = pctx.enter_context(tc.tile_pool(name="p2", bufs=2))
                t2p = pctx.enter_context(tc.tile_pool(name="t2", bufs=2))
                y2p = pctx.enter_context(tc.tile_pool(name="y2", bufs=2))
                y3p = pctx.enter_context(tc.tile_pool(name="y3", bufs=2))
                osbp = pctx.enter_context(tc.tile_pool(name="osb", bufs=2))
                e1p = pctx.enter_context(tc.tile_pool(name="e1", bufs=2))
                p1bp = pctx.enter_context(tc.tile_pool(name="p1b", bufs=2))
                p2bp = pctx.enter_context(tc.tile_pool(name="p2b", bufs=2))
                e2p = pctx.enter_context(tc.tile_pool(name="e2", bufs=2))
                ps1p = pctx.enter_context(tc.tile_pool(name="ps1", bufs=2, space="PSUM"))
                ps2p = pctx.enter_context(tc.tile_pool(name="ps2", bufs=2, space="PSUM"))
                ps3p = pctx.enter_context(tc.tile_pool(name="ps3", bufs=2, space="PSUM"))
                ps4p = pctx.enter_context(tc.tile_pool(name="ps4", bufs=2, space="PSUM"))
                # --- constants ---
                L1 = cpool.tile([9, 520], F16)
                nc.sync.dma_start(out=L1[:, :], in_=L1_d[:, :])
                L2h = cpool.tile([121, 570], F16)
                nc.sync.dma_start(out=L2h[:, :], in_=L2_d[:, :])
                L2 = cpool.tile([121, 570], F32R)
                nc.scalar.copy(out=L2[:, :], in_=L2h[:, :])
                LF1h = cpool.tile([51, 8000], F16)
                nc.sync.dma_start(out=LF1h[:, :], in_=LF1_d[:, :])
                LF1 = cpool.tile([51, 8000], F32R)
                nc.scalar.copy(out=LF1[:, :], in_=LF1h[:, :])
                LF2 = cpool.tile([126, 40], F32R)
                nc.sync.dma_start(out=LF2[:, :], in_=LF2_d[:, :])

                pl1 = _pitch(L1[:, :])
                y2_cur = None
                c2r_tiles = []
                for j in range(2):
                    t_ = c2rp.tile([121, CH * 48], F32R)
                    nc.sync.dma_start(
                        out=_ap(t_[:, :], 120 * _pitch(t_[:, :]),
                                [[_pitch(t_[:, :]), 1], [1, CH * 48]]),
                        in_=_ap(ON_d[:], 0, [[0, 1], [1, CH * 48]]),
                    )
                    c2r_tiles.append(t_)
                xs_tiles = []
                for j in range(2):
                    t_ = xsp.tile([9, CH * 168], F16)
                    nc.sync.dma_start(
                        out=_ap(t_[:, :], 8 * _pitch(t_[:, :]),
                                [[_pitch(t_[:, :]), 1], [1, CH * 168]]),
                        in_=ONH_d[:],
                    )
                    xs_tiles.append(t_)
                for i in range(NCH):
                    q = i // QC
                    # ---- conv1 rhs: on-device im2col-lite (6 strided DMAs) ----
                    XS = xs_tiles[i % 2]
                    px = _pitch(XS[:, :])
                    for g in range(6):
                        nc.sync.dma_start(
                            out=_ap(XS[:, :], g * 28,
                                    [[px, 8], [168, CH], [1, 28]]),
                            in_=_ap(x_d[:, :], i * CH * 784 + g * 112,
                                    [[28, 8], [784, CH], [1, 28]]),
                        )
                    # ---- conv1 matmuls (5 dx-accumulated) + evict + pool-x ----
                    P1 = p1p.tile([104, CH * 72], F32R)
                    pp1 = _pitch(P1[:, :])
                    for bs in range(CH // 2):
                        ps1 = ps1p.tile([104, 288], F32)
                        for dx in range(5):
                            nc.tensor.matmul(
                                ps1[:, :],
                                _ap(L1[:, :], dx * 104, [[pl1, 9], [1, 104]]),
                                _ap(XS[:, :], bs * 336 + dx,
                                    [[px, 9], [168, 2], [28, 6], [1, 24]]),
                                start=(dx == 0), stop=(dx == 4),
                            )
                        E1 = e1p.tile([104, 288], F32)
                        pe1 = _pitch(E1[:, :])
                        nc.scalar.copy(out=E1[:, :], in_=ps1[:, :])
                        nc.vector.tensor_tensor(
                            out=_ap(P1[:, :], bs * 144,
                                    [[pp1, 104], [72, 2], [12, 6], [1, 12]]),
                            in0=_ap(E1[:, :], 0,
                                    [[pe1, 104], [144, 2], [24, 6], [2, 12]]),
                            in1=_ap(E1[:, :], 1,
                                    [[pe1, 104], [144, 2], [24, 6], [2, 12]]),
                            op=MAX,
                        )
                    # ---- conv1 pool-y + relu ----
                    P1B = p1bp.tile([40, CH * 72], F32R)
                    nc.sync.dma_start(out=P1B[:, :], in_=P1[64:104, :])
                    Y1 = y1p.tile([40, CH * 72], F32R)
                    nc.vector.tensor_tensor(
                        out=Y1[:, :], in0=P1[0:40, :], in1=P1B[:, :], op=MAX)
                    nc.vector.tensor_scalar_max(out=Y1[:, :], in0=Y1[:, :],
                                                scalar1=0.0)
                    # ---- shuffle Y1 -> C2R (6 DMAs) ----
                    C2R = c2r_tiles[i % 2]
                    pc = _pitch(C2R[:, :])
                    py1 = _pitch(Y1[:, :])
                    for u in range(2):
                        for v in range(3):
                            nc.sync.dma_start(
                                out=_ap(C2R[:, :], (2 * v + u) * 20 * pc,
                                        [[pc, 20], [48, CH], [1, 48]]),
                                in_=_ap(Y1[:, :], u * 20 * py1 + v * 12,
                                        [[py1, 20], [72, CH], [1, 48]]),
                            )
                    # ---- conv2: groups of 16 samples ----
                    P2 = p2p.tile([114, CH * 16], F32R)
                    pp2 = _pitch(P2[:, :])
                    for bg in range(CH // 16):
                        ps2 = ps2p.tile([114, 512], F32)
                        pq = _pitch(ps2[:, :])
                        for dx in range(5):
                            nc.tensor.matmul(
                                ps2[:, :],
                                _ap(L2[:, :], dx * 114,
                                    [[_pitch(L2[:, :]), 121], [1, 114]]),
                                _ap(C2R[:, :], bg * 16 * 48 + dx,
                                    [[pc, 121], [48, 16], [12, 4], [1, 8]]),
                                start=(dx == 0), stop=(dx == 4),
                            )
                        E2 = e2p.tile([114, 512], F32)
                        pe2 = _pitch(E2[:, :])
                        nc.scalar.copy(out=E2[:, :], in_=ps2[:, :])
                        nc.vector.tensor_tensor(
                            out=_ap(P2[:, :], bg * 256,
                                    [[pp2, 114], [16, 16], [4, 4], [1, 4]]),
                            in0=_ap(E2[:, :], 0,
                                    [[pe2, 114], [32, 16], [8, 4], [2, 4]]),
                            in1=_ap(E2[:, :], 1,
                                    [[pe2, 114], [32, 16], [8, 4], [2, 4]]),
                            op=MAX,
                        )
                    # ---- conv2 pool-y + bias/relu into Y2 ----
                    P2B = p2bp.tile([50, CH * 16], F32R)
                    nc.sync.dma_start(out=P2B[:, :], in_=P2[64:114, :])
                    T2 = t2p.tile([50, CH * 16], F32R)
                    nc.vector.tensor_tensor(
                        out=T2[:, :], in0=P2[0:50, :], in1=P2B[:, :], op=MAX)
                    if i % QC == 0:
                        y2_cur = y2p.tile([51, QC * CH * 16], F32R)
                        nc.sync.dma_start(
                            out=_ap(y2_cur[:, :], 50 * _pitch(y2_cur[:, :]),
                                    [[_pitch(y2_cur[:, :]), 1], [1, QC * CH * 16]]),
                            in_=_ap(ON_d[:], 0, [[0, 1], [1, QC * CH * 16]]),
                        )
                    Y2 = y2_cur
                    nc.vector.tensor_scalar_max(
                        out=Y2[0:50, (i % QC) * CH * 16:(i % QC + 1) * CH * 16],
                        in0=T2[:, :], scalar1=0.0,
                    )
                    # ---- fc1 + fc2 per completed 256-sample group ----
                    if i % QC == QC - 1:
                        NB = QC * CH  # 256
                        py2 = _pitch(Y2[:, :])
                        Y3 = y3p.tile([126, 4 * NB], F32R)
                        nc.sync.dma_start(
                            out=_ap(Y3[:, :], 125 * _pitch(Y3[:, :]),
                                    [[_pitch(Y3[:, :]), 1], [1, 4 * NB]]),
                            in_=_ap(ON_d[:], 0, [[0, 1], [1, 4 * NB]]),
                        )
                        for c in range(4):
                            ps3 = ps3p.tile([125, NB], F32)
                            for f in range(16):
                                nc.tensor.matmul(
                                    ps3[:, :],
                                    _ap(LF1[:, :], f * 500 + c * 125,
                                        [[_pitch(LF1[:, :]), 51], [1, 125]]),
                                    _ap(Y2[:, :], f, [[py2, 51], [16, NB]]),
                                    start=(f == 0), stop=(f == 15),
                                )
                            nc.vector.tensor_scalar_max(
                                out=Y3[0:125, c * NB:(c + 1) * NB],
                                in0=ps3[:, :], scalar1=0.0,
                            )
                        ps4 = ps4p.tile([10, NB], F32)
                        for c in range(4):
                            nc.tensor.matmul(
                                ps4[:, :],
                                _ap(LF2[:, :], c * 10,
                                    [[_pitch(LF2[:, :]), 126], [1, 10]]),
                                _ap(Y3[:, :], c * NB,
                                    [[_pitch(Y3[:, :]), 126], [1, NB]]),
                                start=(c == 0), stop=(c == 3),
                            )
                        OUT = osbp.tile([10, NB], F32)
                        nc.vector.tensor_copy(out=OUT[:, :], in_=ps4[:, :])
                        nc.sync.dma_start(
                            out=_ap(out_d[:], q * NB * 10, [[1, 10], [10, NB]]),
                            in_=_ap(OUT[:, :], 0, [[_pitch(OUT[:, :]), 10], [1, NB]]),
                        )
    return nc


_NC_CACHE = None


def kernel(x, ps, conv1_w, conv1_b, conv2_w, conv2_b, fc1_w, fc1_b, fc2_w, fc2_b):
    global _NC_CACHE
    from concourse import bass_utils

    if _NC_CACHE is None:
        _NC_CACHE = build_nc()
        _NC_CACHE.finalize()
    nc = _NC_CACHE

    in_maps = build_in_maps(x, ps, conv1_w, conv1_b, conv2_w, conv2_b,
                            fc1_w, fc1_b, fc2_w, fc2_b)
    res = bass_utils.run_bass_kernel_spmd(nc, in_maps, core_ids=list(range(NCORES)))
    out = np.concatenate([r["out"] for r in res.results], axis=0)
    return out.astype(np.float32)
